# revision 22
# baseline (speedup 1.0000x reference)
"""Trainium2 Bass kernel for a CLIP encoder layer (B=32, S=257, E=1024, H=16, I=4096).

Strategy: data-parallel over batch across 8 NeuronCores (4 batch elements per
core), no collectives.  Per-core compute is feature-major ([E, tokens]).

Quantization: Q/K/V/O/fc1 matmuls run in fp8(e4m3) with DoubleRow perf mode;
fc2 stays bf16 (fp8 there blows the error budget).  All fp8 weight scales are
powers of two so every dequant factor is exact in bf16/f32.

Scheduling (v2):
  - Activations are stored with a padded per-batch pitch (PT=264 tokens) so
    every per-batch DVE slice is 4B-aligned and runs in 2x mode.
  - LayerNorm is stage-major across the 4 batches (squares for all batches
    first, then stats, rows, normalize) so DVE/ACT/PE pipeline instead of
    serializing a per-batch chain.  Squares run on GpSimd (otherwise idle).
  - LN stats (sum + sum-of-squares x 2 batches) pack into one PSUM bank via
    tile_position column groups -> 4 concurrent subarray matmul streams.
  - Mean/rstd broadcast rows are copied PSUM->SBUF (ACT) so the normalize
    tensor_tensor ops avoid the 1x-mode PSUM penalty.
  - The attention residual add is folded into the o_proj matmul with a
    scaled-identity lhsT tile (exact: power-of-2 scales), so the o_proj
    epilogue is a single ACT dequant into ht3.
  - fc1 runs in two batch-half passes; the first starts as soon as LN2 of
    batches 0/1 is done, keeping the PE dense through the LN2 region so the
    HAM clock gate stays at 8/8.
"""

import numpy as np
import ml_dtypes

B, S, E, H, D, II = 32, 257, 1024, 16, 64, 4096
N_CORES = 8
B_LOC = B // N_CORES          # 4
NT = B_LOC * S                # 1028
PT = 264                      # padded per-batch token pitch (4B-aligned bf16)
NTP = B_LOC * PT              # 1056
KC = E // 128                 # 8
MC_E = E // 128               # 8
MC_I = II // 128              # 32
EPS = 1e-5
SP = 272                      # S padded so fp8 k-slice stride is 16B-aligned

S_X = 16.0                    # LN output fp8 scale
S_CTX = 128.0                 # ctx fp8 scale

# j-chunks of one batch element's 257 keys
JC = [(0, 128), (128, 128), (256, 1)]

TRACE = False
LAST_EXEC_NS = None

_cache = {}


def _build(with_mask, with_vbias, with_qkbias, with_obias,
           dq_v, dq_qk, dq_o, dq_f1):
    import concourse.tile as tile
    from concourse import bacc, mybir
    from contextlib import ExitStack

    F32 = mybir.dt.float32
    BF16 = mybir.dt.bfloat16
    FP8 = mybir.dt.float8e4
    AF = mybir.ActivationFunctionType
    ALU = mybir.AluOpType
    DR = mybir.MatmulPerfMode.DoubleRow

    nc = bacc.Bacc("TRN2", target_bir_lowering=False, debug=False,
                   enable_asserts=False, num_devices=N_CORES)

    # partition-major packing: each SBUF partition row is ONE contiguous DRAM
    # run, so a whole-tensor DMA is 128 descriptors instead of 128/transfer
    xTb_d = nc.dram_tensor("xTb", [128, B_LOC, KC, PT], BF16,
                           kind="ExternalInput")
    qw_d = nc.dram_tensor("qw", [128, MC_E, KC, 128], FP8, kind="ExternalInput")
    kw_d = nc.dram_tensor("kw", [128, MC_E, KC, 128], FP8, kind="ExternalInput")
    vw_d = nc.dram_tensor("vw", [128, KC // 2, 2, E], FP8, kind="ExternalInput")
    ow_d = nc.dram_tensor("ow", [128, MC_E, KC, 128], FP8, kind="ExternalInput")
    f1w_d = nc.dram_tensor("f1w", [128, MC_I, KC, 128], FP8,
                           kind="ExternalInput")
    f2w_d = nc.dram_tensor("f2w", [MC_E, 128, MC_I, 128], BF16, kind="ExternalInput")
    id_d = nc.dram_tensor("ident", [128, 128], BF16, kind="ExternalInput")
    qb_d = nc.dram_tensor("qb", [128, MC_E], F32, kind="ExternalInput")
    kb_d = nc.dram_tensor("kb", [128, MC_E], F32, kind="ExternalInput")
    vb_d = nc.dram_tensor("vb", [1, E], F32, kind="ExternalInput")
    ob_d = nc.dram_tensor("ob", [128, MC_E], F32, kind="ExternalInput")
    f1b_d = nc.dram_tensor("f1b", [128, MC_I], F32, kind="ExternalInput")
    f2b_d = nc.dram_tensor("f2b", [128, MC_E], F32, kind="ExternalInput")
    mskT_d = None
    if with_mask:
        mskT_d = nc.dram_tensor("mskT", [B_LOC, S, S], F32, kind="ExternalInput")
    outT_d = nc.dram_tensor("outT", [E, NT], F32, kind="ExternalOutput")

    with tile.TileContext(nc) as tc, ExitStack() as top:
        consts = top.enter_context(tc.tile_pool(name="consts", bufs=1))

        ones_col = consts.tile([128, 1], BF16)
        nc.vector.memset(ones_col[:], 1.0)
        ones_row = consts.tile([1, 128], BF16)
        nc.vector.memset(ones_row[:], 1.0)
        eps_t = consts.tile([1, 1], F32)
        nc.vector.memset(eps_t[:], EPS)
        spin_x = consts.tile([128, 64], BF16)
        nc.vector.memset(spin_x[:], 0.0)
        ident_sb = consts.tile([128, 128], BF16)
        nc.scalar.dma_start(out=ident_sb[:], in_=id_d[:])
        qb_sb = consts.tile([128, MC_E], F32)
        nc.scalar.dma_start(out=qb_sb[:], in_=qb_d[:])
        kb_sb = consts.tile([128, MC_E], F32)
        nc.scalar.dma_start(out=kb_sb[:], in_=kb_d[:])
        ob_sb = consts.tile([128, MC_E], F32)
        nc.scalar.dma_start(out=ob_sb[:], in_=ob_d[:])
        f2b_sb = consts.tile([128, MC_E], F32)
        nc.scalar.dma_start(out=f2b_sb[:], in_=f2b_d[:])
        f1b_sb = consts.tile([128, MC_I], F32)
        nc.scalar.dma_start(out=f1b_sb[:], in_=f1b_d[:])
        vb_sb = None
        if with_vbias:
            vb_sb = consts.tile([128, E], F32)
            nc.scalar.dma_start(out=vb_sb[:],
                                in_=vb_d[0:1, :].to_broadcast((128, E)))

        def emit_spin(spin_t, n):
            """Dependency-free PE matmuls to pre-warm the HAM clock gate."""
            for _ in range(n):
                nc.tensor.matmul(spin_t[0:1, 0:64], ones_col[:], spin_x[:],
                                 start=True, stop=True)

        # xTb3 lives through o_proj (residual source read by the identity MM);
        # b-major so each batch is one contiguous 128-descriptor DMA
        xtb_p = top.enter_context(tc.tile_pool(name="xtb", bufs=1))
        xTb3 = xtb_p.tile([128, B_LOC, KC, PT], BF16, name="xTb3", tag="xTb3")

        def emit_ln_stats(ph, src_of, sq_of, sfx):
            """Pair-packed LN stats: for each batch pair, one PSUM bank holds
            sum(b_lo)@p0, sumsq(b_lo)@p32, sum(b_hi)@p64, sumsq(b_hi)@p96 via
            tile_position column groups (4 concurrent subarray streams).
            Returns per-batch (st_tile, sum_row, sq_row) triples."""
            pstat = ph.enter_context(
                tc.tile_pool(name=f"pstat{sfx}", bufs=2, space="PSUM"))
            out = []
            for pair in range(B_LOC // 2):
                st = pstat.tile([128, 512], F32, name="st", tag="stat")
                blo, bhi = 2 * pair, 2 * pair + 1
                for k in range(KC):
                    nc.tensor.matmul(st[0:1, 0:S], ones_col[:],
                                     src_of(blo, k),
                                     start=(k == 0), stop=(k == KC - 1))
                    nc.tensor.matmul(st[32:33, 0:S], ones_col[:],
                                     sq_of(blo, k),
                                     start=(k == 0), stop=(k == KC - 1),
                                     tile_position=(0, 32))
                    nc.tensor.matmul(st[64:65, 0:S], ones_col[:],
                                     src_of(bhi, k),
                                     start=(k == 0), stop=(k == KC - 1),
                                     tile_position=(0, 64))
                    nc.tensor.matmul(st[96:97, 0:S], ones_col[:],
                                     sq_of(bhi, k),
                                     start=(k == 0), stop=(k == KC - 1),
                                     tile_position=(0, 96))
                out.append((st, 0, 32))
                out.append((st, 64, 96))
            return out

        def emit_ln_rows(rows, st3):
            """Per-batch scalar rows from packed stats: returns
            (muneg_b bf16, rstdb bf16) row tiles ([1, S])."""
            st, prow, qrow = st3
            musq = rows.tile([1, S], F32, name="musq", tag="row")
            nc.scalar.activation(out=musq[0:1, :], in_=st[prow:prow + 1, 0:S],
                                 func=AF.Square, scale=-1.0 / E)
            muneg_b = rows.tile([1, S], BF16, name="muneg_b", tag="row")
            nc.scalar.mul(out=muneg_b[0:1, :], in_=st[prow:prow + 1, 0:S],
                          mul=-1.0 / E)
            var = rows.tile([1, S], F32, name="var", tag="row")
            nc.vector.scalar_tensor_tensor(
                out=var[0:1, :], in0=st[qrow:qrow + 1, 0:S], scalar=1.0 / E,
                in1=musq[0:1, :], op0=ALU.mult, op1=ALU.subtract)
            sd = rows.tile([1, S], F32, name="sd", tag="row")
            nc.scalar.activation(out=sd[0:1, :], in_=var[0:1, :],
                                 func=AF.Sqrt, bias=eps_t[0:1, 0:1])
            rstd = rows.tile([1, S], F32, name="rstd", tag="row")
            nc.vector.reciprocal_approx_fast(out=rstd[0:1, :],
                                             in_=sd[0:1, :])
            rstdb = rows.tile([1, S], BF16, name="rstdb", tag="row")
            nc.scalar.mul(out=rstdb[0:1, :], in_=rstd[0:1, :], mul=S_X)
            return muneg_b, rstdb

        def emit_ln_bcast(pbc, rsb, muneg_b, rstdb):
            """Broadcast the two rows across partitions (PE) and land them in
            SBUF bf16 (ACT copy) so the normalize ops run in DVE 2x mode."""
            psA = pbc.tile([128, 512], F32, name="psA", tag="bc")
            nc.tensor.matmul(psA[:, 0:S], ones_row[0:1, :], rstdb[0:1, :],
                             start=True, stop=True)
            R_sb = rsb.tile([128, 1, S], BF16, name="R_sb", tag="rsb")
            nc.scalar.copy(out=R_sb[:, 0, :], in_=psA[:, 0:S])
            psB = pbc.tile([128, 512], F32, name="psB", tag="bc")
            nc.tensor.matmul(psB[:, 0:S], ones_row[0:1, :], muneg_b[0:1, :],
                             start=True, stop=True)
            M_sb = rsb.tile([128, 1, S], BF16, name="M_sb", tag="rsb")
            nc.scalar.copy(out=M_sb[:, 0, :], in_=psB[:, 0:S])
            return M_sb, R_sb

        xln2_p = top.enter_context(tc.tile_pool(name="xln2", bufs=B_LOC))
        xln2_3 = [xln2_p.tile([128, KC, SP], FP8, tag="x3b", name="x3b")
                  for _ in range(B_LOC)]

        ht_p = top.enter_context(
            tc.tile_pool(name="ht3", bufs=1, side="right"))
        ht3 = ht_p.tile([128, KC, B_LOC, PT], BF16, name="ht3", tag="ht3")

        with tc.tile_pool(name="ctx3", bufs=B_LOC) as ctx_p:
            ctx3 = [ctx_p.tile([128, MC_E, SP], FP8, tag="ctx3", name="ctx3")
                    for _ in range(B_LOC)]

            # ============= LN1 (+V interleaved per batch) ===============
            with tc.tile_pool(name="x3", bufs=B_LOC) as x3_p, \
                    tc.tile_pool(name="vpool", bufs=9) as v_p:
                x3 = [x3_p.tile([128, KC, SP], FP8, tag="x3", name="x3")
                      for _ in range(B_LOC)]
                with ExitStack() as ln1_ph:
                    # head spins in a transient PSUM pool (closed before the
                    # LN pools open so the bank budget stays <= 8)
                    with tc.tile_pool(name="spin1", bufs=1,
                                      space="PSUM") as spin_p1:
                        spin_t1 = spin_p1.tile([1, 512], F32, name="spin",
                                               tag="spin")
                        emit_spin(spin_t1, 120)

                    vw_p = ln1_ph.enter_context(tc.tile_pool(name="vw", bufs=1))
                    # x DMA per batch (128 contiguous descriptors each),
                    # split over the two independent HWDGE rings
                    for b in range(B_LOC):
                        eng = nc.sync if (b % 2 == 0) else nc.scalar
                        eng.dma_start(out=xTb3[:, b, :, :],
                                      in_=xTb_d[:, b, :, :])
                    # V weights on the SWDGE (gpsimd) queue, off both rings
                    vw_sb = vw_p.tile([128, KC // 2, 2, E], FP8, name="vwk",
                                      tag="vwk")
                    nc.gpsimd.dma_start(out=vw_sb[:], in_=vw_d[:])

                    v_tiles = {}
                    for b in range(B_LOC):
                        for jc in range(2):
                            vt = v_p.tile([128, H, 128], BF16, name="vt",
                                          tag="vt")
                            v_tiles[(b, jc)] = vt
                    vt_t = v_p.tile([128, H, 128], BF16, name="vt_t", tag="vt")
                    for b in range(B_LOC):
                        v_tiles[(b, 2)] = vt_t

                    xtail_p = ln1_ph.enter_context(
                        tc.tile_pool(name="xtail", bufs=1))
                    xtail = xtail_p.tile([128, KC, 112], FP8, name="xtail",
                                         tag="xtail")
                    nc.vector.memset(xtail[:], 0.0)

                    sq_p = ln1_ph.enter_context(tc.tile_pool(name="sqp1",
                                                             bufs=B_LOC))
                    rows = ln1_ph.enter_context(tc.tile_pool(name="rows1",
                                                             bufs=24))
                    rsb_p = ln1_ph.enter_context(tc.tile_pool(name="rsb1",
                                                              bufs=4))
                    lntmp = ln1_ph.enter_context(tc.tile_pool(name="lntmp1",
                                                              bufs=2))
                    pbc = ln1_ph.enter_context(
                        tc.tile_pool(name="pbc1", bufs=2, space="PSUM"))
                    ppv = ln1_ph.enter_context(
                        tc.tile_pool(name="ppv", bufs=2, space="PSUM"))

                    # stage 1: squares (DVE 2x: aligned bf16 slices)
                    sqb = []
                    for b in range(B_LOC):
                        sq = sq_p.tile([128, KC, S], BF16, name="sqb",
                                       tag="sqb")
                        nc.vector.tensor_mul(out=sq[:],
                                             in0=xTb3[:, b, :, 0:S],
                                             in1=xTb3[:, b, :, 0:S])
                        sqb.append(sq)
                    # v_tiles 1/S_CTX columns (GpSimd, after squares)
                    for b in range(B_LOC):
                        for jc in range(2):
                            nc.gpsimd.memset(v_tiles[(b, jc)][:, :, 0:64],
                                             1.0 / S_CTX)
                    nc.gpsimd.memset(vt_t[:, :, 0:64], 1.0 / S_CTX)

                    # stage 2: packed stats
                    st3s = emit_ln_stats(
                        ln1_ph,
                        lambda b, k: xTb3[:, b, k, 0:S],
                        lambda b, k: sqb[b][:, k, :], "1")
                    # stage 3: rows
                    rws = [emit_ln_rows(rows, st3s[b]) for b in range(B_LOC)]

                    def v_proj(b):
                        for jc, (j0, jcs) in enumerate(JC[:2]):
                            ps = ppv.tile([128, 2, 512], F32,
                                          name="vps", tag="vps")
                            for kp in range(KC // 2):
                                for n in range(2):
                                    nc.tensor.matmul(
                                        ps[0:jcs, n, :],
                                        x3[b][:, 2 * kp:2 * kp + 2,
                                              j0:j0 + jcs],
                                        vw_sb[:, kp, :, n * 512:(n + 1) * 512],
                                        start=(kp == 0), stop=(kp == 3),
                                        perf_mode=DR)
                            vt = v_tiles[(b, jc)]
                            if with_vbias:
                                nc.vector.scalar_tensor_tensor(
                                    out=vt[0:jcs, :, 64:128],
                                    in0=ps[0:jcs, :, :], scalar=dq_v,
                                    in1=vb_sb[0:jcs, :],
                                    op0=ALU.mult, op1=ALU.add)
                            else:
                                nc.scalar.mul(out=vt[0:jcs, :, 64:128],
                                              in_=ps[0:jcs, :, :], mul=dq_v)
                        nc.vector.tensor_copy(
                            out=xtail[:, :, 32 * b:32 * b + 1],
                            in_=x3[b][:, :, 256:257])

                    # stage 4 per batch: bcast -> normalize -> V
                    # (the fp8-out mul is 1x mode on DVE; split per k-pair so
                    # the V DoubleRow matmuls start on early k-pairs)
                    for b in range(B_LOC):
                        M_sb, R_sb = emit_ln_bcast(pbc, rsb_p, *rws[b])
                        tmp = lntmp.tile([128, KC, S], BF16, name="tmp",
                                         tag="ap")
                        nc.vector.tensor_add(
                            out=tmp[:], in0=xTb3[:, b, :, 0:S],
                            in1=M_sb[:, 0:1, :].broadcast_to((128, KC, S)))
                        for kp in range(KC // 2):
                            nc.vector.tensor_mul(
                                out=x3[b][:, 2 * kp:2 * kp + 2, 0:S],
                                in0=tmp[:, 2 * kp:2 * kp + 2, :],
                                in1=R_sb[:, 0:1, :]
                                .broadcast_to((128, 2, S)))
                        v_proj(b)

                    # the 4 batches' tail token (j=256), packed col groups
                    ps = ppv.tile([128, 2, 512], F32, name="vps_t", tag="vps")
                    for kp in range(KC // 2):
                        for n in range(2):
                            nc.tensor.matmul(
                                ps[0:97, n, :],
                                xtail[:, 2 * kp:2 * kp + 2, 0:97],
                                vw_sb[:, kp, :, n * 512:(n + 1) * 512],
                                start=(kp == 0), stop=(kp == 3),
                                perf_mode=DR)
                    if with_vbias:
                        nc.vector.scalar_tensor_tensor(
                            out=vt_t[0:97, :, 64:128],
                            in0=ps[0:97, :, :], scalar=dq_v,
                            in1=vb_sb[0:97, :], op0=ALU.mult, op1=ALU.add)
                    else:
                        nc.vector.tensor_scalar_mul(
                            out=vt_t[0:97, :, 64:128],
                            in0=ps[0:97, :, :], scalar1=dq_v)

                # ========= Q/K + attention (per head-pair chunk) =====
                with ExitStack() as ph:
                    qt_p = ph.enter_context(tc.tile_pool(name="qt", bufs=2))
                    kt_p = ph.enter_context(tc.tile_pool(name="kt", bufs=2))
                    wqk_p = ph.enter_context(
                        tc.tile_pool(name="wqk", bufs=2))
                    qw_sb = wqk_p.tile([128, MC_E, KC, 128], FP8,
                                       name="qw_sb", tag="wqk")
                    nc.scalar.dma_start(out=qw_sb[:], in_=qw_d[:])
                    kw_sb = wqk_p.tile([128, MC_E, KC, 128], FP8,
                                       name="kw_sb", tag="wqk")
                    nc.scalar.dma_start(out=kw_sb[:], in_=kw_d[:])
                    e_p = ph.enter_context(tc.tile_pool(name="ep", bufs=9))
                    rs_p = ph.enter_context(tc.tile_pool(name="rsp", bufs=4))
                    if with_mask:
                        msk_p = ph.enter_context(
                            tc.tile_pool(name="mskp", bufs=3 * B_LOC))
                    pp2 = ph.enter_context(
                        tc.tile_pool(name="pp2", bufs=2, space="PSUM"))
                    psp = ph.enter_context(
                        tc.tile_pool(name="psp", bufs=3, space="PSUM"))
                    if with_mask:
                        msk = {}
                        for b in range(B_LOC):
                            for jc, (j0, jcs) in enumerate(JC):
                                mt = msk_p.tile([128, S], F32, name="mt",
                                                tag="mt")
                                nc.sync.dma_start(
                                    out=mt[0:jcs, :],
                                    in_=mskT_d[b, j0:j0 + jcs, :])
                                msk[(b, jc)] = mt

                    # software-pipelined: QK projection chunks of ec+1 are
                    # emitted BETWEEN the softmax groups of ec so the PE
                    # always has dense matmul work during exp/copy waits
                    # (keeps the HAM clock gate at 8/8 through attention)
                    qte_d, kte_d, ett_d = {}, {}, {}

                    def qk_chunk(ec, ci):
                        proj, half = divmod(ci, 2)
                        w_sb, b_sb, opool, otd = (
                            (qw_sb, qb_sb, qt_p, qte_d) if proj == 0
                            else (kw_sb, kb_sb, kt_p, kte_d))
                        if half == 0:
                            otd[ec] = opool.tile([128, B_LOC, PT], BF16,
                                                 name="qk", tag="qk")
                        ot = otd[ec]
                        pss = [pp2.tile([128, 512], F32,
                                        name="pqk", tag="pqk")
                               for _ in range(2)]
                        for kp in range(KC // 2):
                            for bb in range(2):
                                b = half * 2 + bb
                                nc.tensor.matmul(
                                    pss[bb][:, 0:S],
                                    w_sb[:, ec, 2 * kp:2 * kp + 2, :],
                                    x3[b][:, 2 * kp:2 * kp + 2, 0:S],
                                    start=(kp == 0), stop=(kp == 3),
                                    perf_mode=DR)
                        for bb in range(2):
                            b = half * 2 + bb
                            if with_qkbias:
                                nc.vector.tensor_scalar_add(
                                    out=ot[:, b, 0:S],
                                    in0=pss[bb][:, 0:S],
                                    scalar1=b_sb[:, ec:ec + 1])
                            elif proj == 1:
                                # K copies on ACT to balance DVE load
                                nc.scalar.copy(out=ot[:, b, 0:S],
                                               in_=pss[bb][:, 0:S])
                            else:
                                nc.vector.tensor_copy(
                                    out=ot[:, b, 0:S],
                                    in_=pss[bb][:, 0:S])

                    def tail_chunk(ec):
                        # tail key (j=256) for all 4 batches: packed into
                        # array col groups 32b / row groups 64*hi.
                        qte, kte = qte_d[ec], kte_d[ec]
                        ps_t = [pp2.tile([128, 512], F32, name="ps_t",
                                         tag="pqk") for _ in range(2)]
                        et_t = [e_p.tile([128, S], BF16, name="et_t",
                                         tag="et") for _ in range(2)]
                        for hi in range(2):
                            p0 = hi * 64
                            for b in range(B_LOC):
                                nc.tensor.matmul(
                                    ps_t[hi][32 * b:32 * b + 1, 0:S],
                                    kte[p0:p0 + 64, b, 256:257],
                                    qte[p0:p0 + 64, b, 0:S],
                                    start=True, stop=True,
                                    tile_position=(p0, 32 * b))
                            if with_mask:
                                for b in range(B_LOC):
                                    nc.vector.tensor_add(
                                        out=ps_t[hi][32 * b:32 * b + 1, 0:S],
                                        in0=ps_t[hi][32 * b:32 * b + 1, 0:S],
                                        in1=msk[(b, 2)][0:1, :])
                            nc.scalar.activation(out=et_t[hi][0:97, :],
                                                 in_=ps_t[hi][0:97, 0:S],
                                                 func=AF.Exp, scale=dq_qk)
                        ett_d[ec] = et_t

                    def sp_chunk(ec, b):
                        qte, kte = qte_d[ec], kte_d[ec]
                        ets = []
                        for jc, (j0, jcs) in enumerate(JC[:2]):
                            sp = psp.tile([128, 2, 512], F32,
                                          name="sp", tag="sp")
                            for hi in range(2):
                                p0 = hi * 64
                                nc.tensor.matmul(
                                    sp[0:jcs, hi, 0:S],
                                    kte[p0:p0 + 64, b, j0:j0 + jcs],
                                    qte[p0:p0 + 64, b, 0:S],
                                    start=True, stop=True)
                            if with_mask:
                                for hi in range(2):
                                    nc.vector.tensor_add(
                                        out=sp[0:jcs, hi, 0:S],
                                        in0=sp[0:jcs, hi, 0:S],
                                        in1=msk[(b, jc)][0:jcs, :])
                            et = e_p.tile([128, 2, S], BF16,
                                          name="et", tag="et2")
                            nc.scalar.activation(
                                out=et[0:jcs, :, :],
                                in_=sp[0:jcs, :, 0:S], func=AF.Exp,
                                scale=dq_qk)
                            ets.append(et)
                        return ets

                    def ctx_chunk(ec, b, ets):
                        et_t = ett_d[ec]
                        cp = psp.tile([128, 2, 512], F32,
                                      name="cp", tag="sp")
                        for hi in range(2):
                            h = 2 * ec + hi
                            for jc, (j0, jcs) in enumerate(JC[:2]):
                                nc.tensor.matmul(
                                    cp[0:128, hi, 0:S],
                                    v_tiles[(b, jc)][0:jcs, h, :],
                                    ets[jc][0:jcs, hi, :],
                                    start=(jc == 0), stop=False)
                            nc.tensor.matmul(
                                cp[0:128, hi, 0:S],
                                v_tiles[(b, 2)][32 * b:32 * b + 1, h, :],
                                et_t[hi][32 * b:32 * b + 1, :],
                                start=False, stop=True,
                                tile_position=(32 * b, 0))
                        rst = rs_p.tile([64, 2, S], F32,
                                        name="rst", tag="rst")
                        nc.vector.reciprocal_approx_fast(
                            out=rst[0:64, :, :],
                            in_=cp[0:64, :, 0:S])
                        for hi in range(2):
                            nc.vector.tensor_mul(
                                out=ctx3[b][hi * 64:hi * 64 + 64,
                                            ec, 0:S],
                                in0=cp[64:128, hi, 0:S],
                                in1=rst[0:64, hi, :])

                    for ci in range(4):
                        qk_chunk(0, ci)
                    tail_chunk(0)
                    for ec in range(MC_E):
                        for b in range(B_LOC):
                            ets = sp_chunk(ec, b)
                            if ec + 1 < MC_E:
                                qk_chunk(ec + 1, b)
                            ctx_chunk(ec, b, ets)
                        if ec + 1 < MC_E:
                            tail_chunk(ec + 1)

            # ==== out projection (+identity residual) fused with LN2; fc1 ====
            f1o_p = top.enter_context(
                tc.tile_pool(name="f1o", bufs=MC_I, side="right"))
            f1o = []
            with ExitStack() as oph:
                wo_p = oph.enter_context(tc.tile_pool(name="wo", bufs=1))
                wf1_p = oph.enter_context(tc.tile_pool(name="wf1", bufs=1))
                sq2_p = oph.enter_context(tc.tile_pool(name="sqp2",
                                                       bufs=B_LOC))
                rows2 = oph.enter_context(tc.tile_pool(name="rows2", bufs=24))
                rsb2_p = oph.enter_context(tc.tile_pool(name="rsb2", bufs=4))
                lntmp2 = oph.enter_context(tc.tile_pool(name="lntmp2",
                                                        bufs=2))
                ppo = oph.enter_context(
                    tc.tile_pool(name="ppo", bufs=2, space="PSUM"))
                pbc2 = oph.enter_context(
                    tc.tile_pool(name="pbc2", bufs=2, space="PSUM"))

                wo_sb = wo_p.tile([128, MC_E, KC, 128], FP8, name="wo",
                                  tag="wo")
                nc.gpsimd.dma_start(out=wo_sb[:], in_=ow_d[:])
                wf1_sb = wf1_p.tile([128, MC_I, KC, 128], FP8, name="wf1",
                                    tag="wf1")
                nc.gpsimd.dma_start(out=wf1_sb[:], in_=f1w_d[:])

                def o_proj(b):
                    for m in range(MC_E):
                        ps = ppo.tile([128, 512], F32, name="po", tag="po")
                        for kp in range(KC // 2):
                            nc.tensor.matmul(
                                ps[:, 0:S],
                                wo_sb[:, m, 2 * kp:2 * kp + 2, :],
                                ctx3[b][:, 2 * kp:2 * kp + 2, 0:S],
                                start=(kp == 0), stop=False,
                                perf_mode=DR)
                        # residual: += (1/dq_o) * I @ x  (exact: dq_o = 2^-k)
                        nc.tensor.matmul(
                            ps[:, 0:S], ident_sb[:], xTb3[:, b, m, 0:S],
                            start=False, stop=True)
                        if with_obias:
                            nc.vector.scalar_tensor_tensor(
                                out=ht3[:, m, b, 0:S], in0=ps[:, 0:S],
                                scalar=dq_o, in1=ob_sb[:, m:m + 1]
                                .broadcast_to((128, S)),
                                op0=ALU.mult, op1=ALU.add)
                        else:
                            nc.scalar.mul(out=ht3[:, m, b, 0:S],
                                          in_=ps[:, 0:S], mul=dq_o)

                def sqb2_emit(b):
                    sq = sq2_p.tile([128, KC, S], BF16, name="sqb2",
                                    tag="sqb2")
                    nc.vector.tensor_mul(out=sq[:], in0=ht3[:, :, b, 0:S],
                                         in1=ht3[:, :, b, 0:S])
                    return sq

                def ln2_norm(b, rws2b):
                    M_sb, R_sb = emit_ln_bcast(pbc2, rsb2_p, *rws2b)
                    tmp = lntmp2.tile([128, KC, S], BF16, name="tmp2",
                                      tag="ap2")
                    nc.vector.tensor_add(
                        out=tmp[:], in0=ht3[:, :, b, 0:S],
                        in1=M_sb[:, 0:1, :].broadcast_to((128, KC, S)))
                    for kp in range(KC // 2):
                        nc.vector.tensor_mul(
                            out=xln2_3[b][:, 2 * kp:2 * kp + 2, 0:S],
                            in0=tmp[:, 2 * kp:2 * kp + 2, :],
                            in1=R_sb[:, 0:1, :].broadcast_to((128, 2, S)))

                def fc1_half(half, ppf1, m_lo=0, m_hi=MC_I):
                    for m in range(m_lo, m_hi):
                        ps = ppf1.tile([128, 2, 512], F32, name="pf1",
                                       tag="pf1")
                        for kp in range(KC // 2):
                            for bb in range(2):
                                b = half * 2 + bb
                                nc.tensor.matmul(
                                    ps[:, bb, 0:S],
                                    wf1_sb[:, m, 2 * kp:2 * kp + 2, :],
                                    xln2_3[b][:, 2 * kp:2 * kp + 2, 0:S],
                                    start=(kp == 0), stop=(kp == 3),
                                    perf_mode=DR)
                        if half == 0:
                            o = f1o_p.tile([128, NT], BF16, name="f1o",
                                           tag="f1o")
                            f1o.append(o)
                        else:
                            o = f1o[m]
                        nc.scalar.activation(
                            out=o[:, half * 2 * S:(half + 1) * 2 * S],
                            in_=ps[:, :, 0:S],
                            func=AF.Gelu_apprx_tanh,
                            bias=f1b_sb[:, m:m + 1],
                            scale=dq_f1)

                # pipeline: o_proj per batch; LN2 stages slotted between;
                # fc1 half-passes as soon as their xln2 batches are ready
                o_proj(0)
                sq0 = sqb2_emit(0)
                o_proj(1)
                sq1 = sqb2_emit(1)
                sqs = {0: sq0, 1: sq1}
                st3s2 = {}
                with ExitStack() as stat2_ph:
                    pstat2 = stat2_ph.enter_context(
                        tc.tile_pool(name="pstat2", bufs=2, space="PSUM"))

                    def stats2(pair):
                        blo, bhi = 2 * pair, 2 * pair + 1
                        st = pstat2.tile([128, 512], F32, name="st2",
                                         tag="stat2")
                        for k in range(KC):
                            nc.tensor.matmul(st[0:1, 0:S], ones_col[:],
                                             ht3[:, k, blo, 0:S],
                                             start=(k == 0), stop=(k == KC - 1))
                            nc.tensor.matmul(st[32:33, 0:S], ones_col[:],
                                             sqs[blo][:, k, :],
                                             start=(k == 0), stop=(k == KC - 1),
                                             tile_position=(0, 32))
                            nc.tensor.matmul(st[64:65, 0:S], ones_col[:],
                                             ht3[:, k, bhi, 0:S],
                                             start=(k == 0), stop=(k == KC - 1),
                                             tile_position=(0, 64))
                            nc.tensor.matmul(st[96:97, 0:S], ones_col[:],
                                             sqs[bhi][:, k, :],
                                             start=(k == 0), stop=(k == KC - 1),
                                             tile_position=(0, 96))
                        st3s2[blo] = (st, 0, 32)
                        st3s2[bhi] = (st, 64, 96)

                    stats2(0)
                    rws2 = {b: emit_ln_rows(rows2, st3s2[b]) for b in (0, 1)}
                    ln2_norm(0, rws2[0])
                    ln2_norm(1, rws2[1])
                    o_proj(2)
                    sqs[2] = sqb2_emit(2)
                    o_proj(3)
                    sqs[3] = sqb2_emit(3)
                    stats2(1)
                    for b in (2, 3):
                        rws2[b] = emit_ln_rows(rows2, st3s2[b])
                # pstat2 closed -> banks free for fc1
                ppf1 = oph.enter_context(
                    tc.tile_pool(name="ppf1", bufs=2, space="PSUM"))
                # first fc1 m-chunk (needs only batches 0/1) keeps the PE
                # fed while the LN2 rows/normalize for batches 2/3 drain
                fc1_half(0, ppf1, 0, 8)
                ln2_norm(2, rws2[2])
                ln2_norm(3, rws2[3])
                fc1_half(0, ppf1, 8, MC_I)
                fc1_half(1, ppf1)
        # ctx3 closed

        # ================= fc2 =====================================
        with ExitStack() as ph:
            wf2_p = ph.enter_context(tc.tile_pool(name="wf2", bufs=3))
            ppf2 = ph.enter_context(
                tc.tile_pool(name="ppf2", bufs=2, space="PSUM"))
            out_p = ph.enter_context(tc.tile_pool(name="outp", bufs=3))
            for m in range(MC_E):
                wt = wf2_p.tile([128, MC_I, 128], BF16, name="wf2", tag="wf2")
                nc.gpsimd.dma_start(out=wt[:], in_=f2w_d[m, :, :, :])
                ps = ppf2.tile([128, B_LOC, 512], F32, name="pf2", tag="pf2")
                for b in range(B_LOC):
                    for k in range(MC_I):
                        nc.tensor.matmul(
                            ps[:, b, 0:S], wt[:, k, :],
                            f1o[k][:, b * S:(b + 1) * S],
                            start=(k == 0), stop=(k == MC_I - 1))
                o = out_p.tile([128, B_LOC, S], F32, name="oo", tag="oo")
                nc.vector.scalar_tensor_tensor(
                    out=o[:], in0=ps[:, :, 0:S], scalar=f2b_sb[:, m:m + 1],
                    in1=ht3[:, m, :, 0:S], op0=ALU.add, op1=ALU.add)
                nc.sync.dma_start(out=outT_d[m * 128:(m + 1) * 128, :],
                                  in_=o[:])

    nc.compile()
    return nc


FP8_NP = ml_dtypes.float8_e4m3fn


def _q8(W, s):
    """Quantize W*s to e4m3 (clipped to TRN max normal 240)."""
    return np.clip(np.asarray(W, np.float32) * s, -240, 240).astype(FP8_NP)


def _pack_lhsT8(W, s):
    """W [M, K] (out, in) -> [128, M/128, K/128, 128] fp8 with
    [p, m, k, j] = W[m*128+j, k*128+p]*s (partition-major lhsT tiles:
    each partition row is one contiguous DRAM run -> 128-descriptor DMA)."""
    W = np.asarray(W, np.float32)
    M, K = W.shape
    A = W.reshape(M // 128, 128, K // 128, 128)
    return _q8(np.ascontiguousarray(A.transpose(3, 0, 2, 1)), s)


def _pack_lhsT(W):
    """bf16 variant of _pack_lhsT8 (no scale)."""
    W = np.asarray(W, np.float32)
    M, K = W.shape
    A = W.reshape(M // 128, 128, K // 128, 128)
    return np.ascontiguousarray(A.transpose(0, 3, 2, 1)).astype(ml_dtypes.bfloat16)


def _pack_pbias(b):
    """b [M] -> [128, M/128] f32 per-partition bias columns."""
    return np.ascontiguousarray(np.asarray(b, np.float32).reshape(-1, 128).T)


def _wscale(W):
    """Power-of-2 scale with max |W*s| in (60, 120]."""
    m = max(np.abs(np.asarray(W, np.float32)).max(), 1e-30)
    return float(2.0 ** np.floor(np.log2(120.0 / m)))


def kernel(hidden_states, attention_mask, causal_attention_mask,
           ln1_w, ln1_b, q_w, q_b, k_w, k_b, v_w, v_b, o_w, o_b,
           ln2_w, ln2_b, fc1_w, fc1_b, fc2_w, fc2_b):
    global LAST_EXEC_NS
    from concourse.bass_utils import run_bass_kernel_spmd

    hs = np.asarray(hidden_states, np.float32)
    msk = (np.asarray(attention_mask, np.float32)
           + np.asarray(causal_attention_mask, np.float32))
    with_mask = bool(np.any(msk))

    ln1_w = np.asarray(ln1_w, np.float32); ln1_b = np.asarray(ln1_b, np.float32)
    ln2_w = np.asarray(ln2_w, np.float32); ln2_b = np.asarray(ln2_b, np.float32)
    q_w = np.asarray(q_w, np.float32); q_b = np.asarray(q_b, np.float32)
    k_w = np.asarray(k_w, np.float32); k_b = np.asarray(k_b, np.float32)
    v_w = np.asarray(v_w, np.float32); v_b = np.asarray(v_b, np.float32)
    o_w = np.asarray(o_w, np.float32); o_b = np.asarray(o_b, np.float32)
    fc1_w = np.asarray(fc1_w, np.float32); fc1_b = np.asarray(fc1_b, np.float32)
    fc2_w = np.asarray(fc2_w, np.float32); fc2_b = np.asarray(fc2_b, np.float32)

    scale = D ** -0.5
    # fold LN1 scale/bias into Q/K/V, and the softmax scale into Q
    qw_eff = (q_w * ln1_w[None, :]) * scale
    qb_eff = (q_b + q_w @ ln1_b) * scale
    kw_eff = k_w * ln1_w[None, :]
    kb_eff = k_b + k_w @ ln1_b
    vw_eff = v_w * ln1_w[None, :]
    vb_eff = v_b + v_w @ ln1_b
    # fold LN2 into fc1
    f1w_eff = fc1_w * ln2_w[None, :]
    f1b_eff = fc1_b + fc1_w @ ln2_b

    # fp8 weight scales (power-of-2; LN activations pre-scaled by S_X)
    s_wq = _wscale(qw_eff)
    s_wk = _wscale(kw_eff)
    s_wv = _wscale(vw_eff)
    s_wo = _wscale(o_w)
    s_wf1 = _wscale(f1w_eff)
    dq_qk = 1.0 / (S_X * S_X * s_wq * s_wk)
    dq_v = 1.0 / (S_X * s_wv)
    dq_o = 1.0 / (S_CTX * s_wo)
    dq_f1 = 1.0 / (S_X * s_wf1)

    # vw: [E_in, E_out] grouped into k-pairs -> [128, KC/2, 2, E] fp8
    vw_t = np.ascontiguousarray(vw_eff.T.reshape(KC, 128, E))
    vw_pk = np.ascontiguousarray(
        vw_t.reshape(KC // 2, 2, 128, E).transpose(2, 0, 1, 3))

    base = {
        "qw": _pack_lhsT8(qw_eff, s_wq),
        "kw": _pack_lhsT8(kw_eff, s_wk),
        "vw": _q8(vw_pk, s_wv),
        "ow": _pack_lhsT8(o_w, s_wo),
        "f1w": _pack_lhsT8(f1w_eff, s_wf1),
        "f2w": _pack_lhsT(fc2_w),
        "ident": np.ascontiguousarray(
            (np.eye(128, dtype=np.float32) / dq_o)
            .astype(ml_dtypes.bfloat16)),
        "qb": _pack_pbias(qb_eff * (S_X * s_wq)),
        "kb": _pack_pbias(kb_eff * (S_X * s_wk)),
        "vb": np.ascontiguousarray(vb_eff[None, :].astype(np.float32)),
        "ob": _pack_pbias(o_b),
        "f1b": _pack_pbias(f1b_eff),
        "f2b": _pack_pbias(fc2_b),
    }

    with_vbias = bool(np.any(vb_eff))
    with_qkbias = bool(np.any(qb_eff)) or bool(np.any(kb_eff))
    with_obias = bool(np.any(o_b))
    key = (with_mask, with_vbias, with_qkbias, with_obias,
           dq_v, dq_qk, dq_o, dq_f1)
    if key not in _cache:
        _cache[key] = _build(with_mask, with_vbias, with_qkbias, with_obias,
                             dq_v, dq_qk, dq_o, dq_f1)
    nc = _cache[key]

    in_maps = []
    for c in range(N_CORES):
        # [128, B_LOC, KC, PT]: partition-major, b-major
        xp = np.zeros((128, B_LOC, KC, PT), np.float32)
        for b in range(B_LOC):
            xb = hs[c * B_LOC + b]                      # [S, E]
            xp[:, b, :, 0:S] = xb.T.reshape(KC, 128, S).transpose(1, 0, 2)
        m = dict(base)
        m["xTb"] = np.ascontiguousarray(xp).astype(ml_dtypes.bfloat16)
        if with_mask:
            m["mskT"] = np.ascontiguousarray(
                msk[c * B_LOC:(c + 1) * B_LOC, 0].transpose(0, 2, 1)
                / dq_qk)
        in_maps.append(m)

    res = run_bass_kernel_spmd(nc, in_maps, core_ids=list(range(N_CORES)),
                               trace=TRACE)
    LAST_EXEC_NS = res.exec_time_ns

    outs = []
    for c in range(N_CORES):
        oT = res.results[c]["outT"]          # [E, NT] f32
        outs.append(np.ascontiguousarray(oT.T).reshape(B_LOC, S, E))
    return np.concatenate(outs, axis=0)


# revision 24
# speedup vs baseline: 1.0175x; 1.0175x over previous
"""Trainium2 Bass kernel for a CLIP encoder layer (B=32, S=257, E=1024, H=16, I=4096).

Strategy: data-parallel over batch across 8 NeuronCores (4 batch elements per
core), no collectives.  Per-core compute is feature-major ([E, tokens]).

Quantization: Q/K/V/O/fc1 matmuls run in fp8(e4m3) with DoubleRow perf mode;
fc2 stays bf16 (fp8 there blows the error budget).  All fp8 weight scales are
powers of two so every dequant factor is exact in bf16/f32.

Scheduling (v2):
  - Activations are stored with a padded per-batch pitch (PT=264 tokens) so
    every per-batch DVE slice is 4B-aligned and runs in 2x mode.
  - LayerNorm is stage-major across the 4 batches (squares for all batches
    first, then stats, rows, normalize) so DVE/ACT/PE pipeline instead of
    serializing a per-batch chain.  Squares run on GpSimd (otherwise idle).
  - LN stats (sum + sum-of-squares x 2 batches) pack into one PSUM bank via
    tile_position column groups -> 4 concurrent subarray matmul streams.
  - Mean/rstd broadcast rows are copied PSUM->SBUF (ACT) so the normalize
    tensor_tensor ops avoid the 1x-mode PSUM penalty.
  - The attention residual add is folded into the o_proj matmul with a
    scaled-identity lhsT tile (exact: power-of-2 scales), so the o_proj
    epilogue is a single ACT dequant into ht3.
  - fc1 runs in two batch-half passes; the first starts as soon as LN2 of
    batches 0/1 is done, keeping the PE dense through the LN2 region so the
    HAM clock gate stays at 8/8.
"""

import numpy as np
import ml_dtypes

B, S, E, H, D, II = 32, 257, 1024, 16, 64, 4096
N_CORES = 8
B_LOC = B // N_CORES          # 4
NT = B_LOC * S                # 1028
PT = 264                      # padded per-batch token pitch (4B-aligned bf16)
NTP = B_LOC * PT              # 1056
KC = E // 128                 # 8
MC_E = E // 128               # 8
MC_I = II // 128              # 32
EPS = 1e-5
SP = 272                      # S padded so fp8 k-slice stride is 16B-aligned

S_X = 16.0                    # LN output fp8 scale
S_CTX = 128.0                 # ctx fp8 scale

# j-chunks of one batch element's 257 keys
JC = [(0, 128), (128, 128), (256, 1)]

TRACE = False
LAST_EXEC_NS = None

_cache = {}


def _build(with_mask, with_vbias, with_qkbias, with_obias,
           dq_v, dq_qk, dq_o, dq_f1):
    import concourse.tile as tile
    from concourse import bacc, mybir
    from contextlib import ExitStack

    F32 = mybir.dt.float32
    BF16 = mybir.dt.bfloat16
    FP8 = mybir.dt.float8e4
    AF = mybir.ActivationFunctionType
    ALU = mybir.AluOpType
    DR = mybir.MatmulPerfMode.DoubleRow

    nc = bacc.Bacc("TRN2", target_bir_lowering=False, debug=False,
                   enable_asserts=False, num_devices=N_CORES)

    # partition-major packing: each SBUF partition row is ONE contiguous DRAM
    # run, so a whole-tensor DMA is 128 descriptors instead of 128/transfer
    xTb_d = nc.dram_tensor("xTb", [128, B_LOC, KC, PT], BF16,
                           kind="ExternalInput")
    qw_d = nc.dram_tensor("qw", [128, MC_E, KC, 128], FP8, kind="ExternalInput")
    kw_d = nc.dram_tensor("kw", [128, MC_E, KC, 128], FP8, kind="ExternalInput")
    vw_d = nc.dram_tensor("vw", [128, KC // 2, 2, E], FP8, kind="ExternalInput")
    ow_d = nc.dram_tensor("ow", [128, MC_E, KC, 128], FP8, kind="ExternalInput")
    f1w_d = nc.dram_tensor("f1w", [128, MC_I, KC, 128], FP8,
                           kind="ExternalInput")
    f2w_d = nc.dram_tensor("f2w", [MC_E, 128, MC_I, 128], BF16, kind="ExternalInput")
    id_d = nc.dram_tensor("ident", [128, 128], BF16, kind="ExternalInput")
    qb_d = nc.dram_tensor("qb", [128, MC_E], F32, kind="ExternalInput")
    kb_d = nc.dram_tensor("kb", [128, MC_E], F32, kind="ExternalInput")
    vb_d = nc.dram_tensor("vb", [1, E], F32, kind="ExternalInput")
    ob_d = nc.dram_tensor("ob", [128, MC_E], F32, kind="ExternalInput")
    f1b_d = nc.dram_tensor("f1b", [128, MC_I], F32, kind="ExternalInput")
    f2b_d = nc.dram_tensor("f2b", [128, MC_E], F32, kind="ExternalInput")
    mskT_d = None
    if with_mask:
        mskT_d = nc.dram_tensor("mskT", [B_LOC, S, S], F32, kind="ExternalInput")
    outT_d = nc.dram_tensor("outT", [E, NT], F32, kind="ExternalOutput")

    with tile.TileContext(nc) as tc, ExitStack() as top:
        consts = top.enter_context(tc.tile_pool(name="consts", bufs=1))

        ones_col = consts.tile([128, 1], BF16)
        nc.vector.memset(ones_col[:], 1.0)
        ones_row = consts.tile([1, 128], BF16)
        nc.vector.memset(ones_row[:], 1.0)
        eps_t = consts.tile([1, 1], F32)
        nc.vector.memset(eps_t[:], EPS)
        spin_x = consts.tile([128, 64], BF16)
        nc.vector.memset(spin_x[:], 0.0)
        ident_sb = consts.tile([128, 128], BF16)
        nc.scalar.dma_start(out=ident_sb[:], in_=id_d[:])
        qb_sb = consts.tile([128, MC_E], F32)
        nc.scalar.dma_start(out=qb_sb[:], in_=qb_d[:])
        kb_sb = consts.tile([128, MC_E], F32)
        nc.scalar.dma_start(out=kb_sb[:], in_=kb_d[:])
        ob_sb = consts.tile([128, MC_E], F32)
        nc.scalar.dma_start(out=ob_sb[:], in_=ob_d[:])
        f2b_sb = consts.tile([128, MC_E], F32)
        nc.scalar.dma_start(out=f2b_sb[:], in_=f2b_d[:])
        f1b_sb = consts.tile([128, MC_I], F32)
        nc.scalar.dma_start(out=f1b_sb[:], in_=f1b_d[:])
        vb_sb = None
        if with_vbias:
            vb_sb = consts.tile([128, E], F32)
            nc.scalar.dma_start(out=vb_sb[:],
                                in_=vb_d[0:1, :].to_broadcast((128, E)))

        def emit_spin(spin_t, n):
            """Dependency-free PE matmuls to pre-warm the HAM clock gate."""
            for _ in range(n):
                nc.tensor.matmul(spin_t[0:1, 0:64], ones_col[:], spin_x[:],
                                 start=True, stop=True)

        # xTb3 lives through o_proj (residual source read by the identity MM);
        # b-major so each batch is one contiguous 128-descriptor DMA
        xtb_p = top.enter_context(tc.tile_pool(name="xtb", bufs=1))
        xTb3 = xtb_p.tile([128, B_LOC, KC, PT], BF16, name="xTb3", tag="xTb3")

        def emit_ln_stats(ph, src_of, sq_of, sfx):
            """Pair-packed LN stats: for each batch pair, one PSUM bank holds
            sum(b_lo)@p0, sumsq(b_lo)@p32, sum(b_hi)@p64, sumsq(b_hi)@p96 via
            tile_position column groups (4 concurrent subarray streams).
            Returns per-batch (st_tile, sum_row, sq_row) triples."""
            pstat = ph.enter_context(
                tc.tile_pool(name=f"pstat{sfx}", bufs=2, space="PSUM"))
            out = []
            for pair in range(B_LOC // 2):
                st = pstat.tile([128, 512], F32, name="st", tag="stat")
                blo, bhi = 2 * pair, 2 * pair + 1
                for k in range(KC):
                    nc.tensor.matmul(st[0:1, 0:S], ones_col[:],
                                     src_of(blo, k),
                                     start=(k == 0), stop=(k == KC - 1))
                    nc.tensor.matmul(st[32:33, 0:S], ones_col[:],
                                     sq_of(blo, k),
                                     start=(k == 0), stop=(k == KC - 1),
                                     tile_position=(0, 32))
                    nc.tensor.matmul(st[64:65, 0:S], ones_col[:],
                                     src_of(bhi, k),
                                     start=(k == 0), stop=(k == KC - 1),
                                     tile_position=(0, 64))
                    nc.tensor.matmul(st[96:97, 0:S], ones_col[:],
                                     sq_of(bhi, k),
                                     start=(k == 0), stop=(k == KC - 1),
                                     tile_position=(0, 96))
                out.append((st, 0, 32))
                out.append((st, 64, 96))
            return out

        def emit_ln_rows(rows, st3):
            """Per-batch scalar rows from packed stats: returns
            (muneg_b bf16, rstdb bf16) row tiles ([1, S])."""
            st, prow, qrow = st3
            musq = rows.tile([1, S], F32, name="musq", tag="row")
            nc.scalar.activation(out=musq[0:1, :], in_=st[prow:prow + 1, 0:S],
                                 func=AF.Square, scale=-1.0 / E)
            muneg_b = rows.tile([1, S], BF16, name="muneg_b", tag="row")
            nc.scalar.mul(out=muneg_b[0:1, :], in_=st[prow:prow + 1, 0:S],
                          mul=-1.0 / E)
            var = rows.tile([1, S], F32, name="var", tag="row")
            nc.vector.scalar_tensor_tensor(
                out=var[0:1, :], in0=st[qrow:qrow + 1, 0:S], scalar=1.0 / E,
                in1=musq[0:1, :], op0=ALU.mult, op1=ALU.subtract)
            sd = rows.tile([1, S], F32, name="sd", tag="row")
            nc.scalar.activation(out=sd[0:1, :], in_=var[0:1, :],
                                 func=AF.Sqrt, bias=eps_t[0:1, 0:1])
            rstd = rows.tile([1, S], F32, name="rstd", tag="row")
            nc.vector.reciprocal_approx_fast(out=rstd[0:1, :],
                                             in_=sd[0:1, :])
            rstdb = rows.tile([1, S], BF16, name="rstdb", tag="row")
            nc.scalar.mul(out=rstdb[0:1, :], in_=rstd[0:1, :], mul=S_X)
            return muneg_b, rstdb

        def emit_ln_bcast(pbc, rsb, muneg_b, rstdb):
            """Broadcast the two rows across partitions (PE) and land them in
            SBUF bf16 (ACT copy) so the normalize ops run in DVE 2x mode."""
            psA = pbc.tile([128, 512], F32, name="psA", tag="bc")
            nc.tensor.matmul(psA[:, 0:S], ones_row[0:1, :], rstdb[0:1, :],
                             start=True, stop=True)
            R_sb = rsb.tile([128, 1, S], BF16, name="R_sb", tag="rsb")
            nc.scalar.copy(out=R_sb[:, 0, :], in_=psA[:, 0:S])
            psB = pbc.tile([128, 512], F32, name="psB", tag="bc")
            nc.tensor.matmul(psB[:, 0:S], ones_row[0:1, :], muneg_b[0:1, :],
                             start=True, stop=True)
            M_sb = rsb.tile([128, 1, S], BF16, name="M_sb", tag="rsb")
            nc.scalar.copy(out=M_sb[:, 0, :], in_=psB[:, 0:S])
            return M_sb, R_sb

        xln2_p = top.enter_context(tc.tile_pool(name="xln2", bufs=B_LOC))
        xln2_3 = [xln2_p.tile([128, KC, SP], FP8, tag="x3b", name="x3b")
                  for _ in range(B_LOC)]

        ht_p = top.enter_context(
            tc.tile_pool(name="ht3", bufs=1, side="right"))
        ht3 = ht_p.tile([128, KC, B_LOC, PT], BF16, name="ht3", tag="ht3")

        with tc.tile_pool(name="ctx3", bufs=B_LOC) as ctx_p:
            ctx3 = [ctx_p.tile([128, MC_E, SP], FP8, tag="ctx3", name="ctx3")
                    for _ in range(B_LOC)]

            # ============= LN1 (+V interleaved per batch) ===============
            with tc.tile_pool(name="x3", bufs=B_LOC) as x3_p, \
                    tc.tile_pool(name="vpool", bufs=9) as v_p:
                x3 = [x3_p.tile([128, KC, SP], FP8, tag="x3", name="x3")
                      for _ in range(B_LOC)]
                with ExitStack() as ln1_ph:
                    # head spins in a transient PSUM pool (closed before the
                    # LN pools open so the bank budget stays <= 8)
                    with tc.tile_pool(name="spin1", bufs=1,
                                      space="PSUM") as spin_p1:
                        spin_t1 = spin_p1.tile([1, 512], F32, name="spin",
                                               tag="spin")
                        emit_spin(spin_t1, 120)

                    vw_p = ln1_ph.enter_context(tc.tile_pool(name="vw", bufs=1))
                    # x DMA per batch (128 contiguous descriptors each),
                    # split over the two independent HWDGE rings
                    for b in range(B_LOC):
                        eng = nc.sync if (b % 2 == 0) else nc.scalar
                        eng.dma_start(out=xTb3[:, b, :, :],
                                      in_=xTb_d[:, b, :, :])
                    # V weights on the SWDGE (gpsimd) queue, off both rings
                    vw_sb = vw_p.tile([128, KC // 2, 2, E], FP8, name="vwk",
                                      tag="vwk")
                    nc.gpsimd.dma_start(out=vw_sb[:], in_=vw_d[:])

                    v_tiles = {}
                    for b in range(B_LOC):
                        for jc in range(2):
                            vt = v_p.tile([128, H, 128], BF16, name="vt",
                                          tag="vt")
                            v_tiles[(b, jc)] = vt
                    vt_t = v_p.tile([128, H, 128], BF16, name="vt_t", tag="vt")
                    for b in range(B_LOC):
                        v_tiles[(b, 2)] = vt_t

                    xtail_p = ln1_ph.enter_context(
                        tc.tile_pool(name="xtail", bufs=1))
                    xtail = xtail_p.tile([128, KC, 112], FP8, name="xtail",
                                         tag="xtail")
                    nc.vector.memset(xtail[:], 0.0)

                    sq_p = ln1_ph.enter_context(tc.tile_pool(name="sqp1",
                                                             bufs=B_LOC))
                    rows = ln1_ph.enter_context(tc.tile_pool(name="rows1",
                                                             bufs=24))
                    rsb_p = ln1_ph.enter_context(tc.tile_pool(name="rsb1",
                                                              bufs=4))
                    lntmp = ln1_ph.enter_context(tc.tile_pool(name="lntmp1",
                                                              bufs=2))
                    pbc = ln1_ph.enter_context(
                        tc.tile_pool(name="pbc1", bufs=2, space="PSUM"))
                    ppv = ln1_ph.enter_context(
                        tc.tile_pool(name="ppv", bufs=2, space="PSUM"))

                    # stage 1: squares (DVE 2x: aligned bf16 slices)
                    sqb = []
                    for b in range(B_LOC):
                        sq = sq_p.tile([128, KC, S], BF16, name="sqb",
                                       tag="sqb")
                        nc.vector.tensor_mul(out=sq[:],
                                             in0=xTb3[:, b, :, 0:S],
                                             in1=xTb3[:, b, :, 0:S])
                        sqb.append(sq)
                    # v_tiles 1/S_CTX columns (GpSimd, after squares)
                    for b in range(B_LOC):
                        for jc in range(2):
                            nc.gpsimd.memset(v_tiles[(b, jc)][:, :, 0:64],
                                             1.0 / S_CTX)
                    nc.gpsimd.memset(vt_t[:, :, 0:64], 1.0 / S_CTX)

                    # stage 2: packed stats
                    st3s = emit_ln_stats(
                        ln1_ph,
                        lambda b, k: xTb3[:, b, k, 0:S],
                        lambda b, k: sqb[b][:, k, :], "1")
                    # stage 3: rows
                    rws = [emit_ln_rows(rows, st3s[b]) for b in range(B_LOC)]

                    def v_proj(b):
                        for jc, (j0, jcs) in enumerate(JC[:2]):
                            ps = ppv.tile([128, 2, 512], F32,
                                          name="vps", tag="vps")
                            for kp in range(KC // 2):
                                for n in range(2):
                                    nc.tensor.matmul(
                                        ps[0:jcs, n, :],
                                        x3[b][:, 2 * kp:2 * kp + 2,
                                              j0:j0 + jcs],
                                        vw_sb[:, kp, :, n * 512:(n + 1) * 512],
                                        start=(kp == 0), stop=(kp == 3),
                                        perf_mode=DR)
                            vt = v_tiles[(b, jc)]
                            if with_vbias:
                                nc.vector.scalar_tensor_tensor(
                                    out=vt[0:jcs, :, 64:128],
                                    in0=ps[0:jcs, :, :], scalar=dq_v,
                                    in1=vb_sb[0:jcs, :],
                                    op0=ALU.mult, op1=ALU.add)
                            else:
                                nc.scalar.mul(out=vt[0:jcs, :, 64:128],
                                              in_=ps[0:jcs, :, :], mul=dq_v)
                        nc.vector.tensor_copy(
                            out=xtail[:, :, 32 * b:32 * b + 1],
                            in_=x3[b][:, :, 256:257])

                    # stage 4 per batch: bcast -> normalize -> V
                    # (the fp8-out mul is 1x mode on DVE; split per k-pair so
                    # the V DoubleRow matmuls start on early k-pairs)
                    for b in range(B_LOC):
                        M_sb, R_sb = emit_ln_bcast(pbc, rsb_p, *rws[b])
                        tmp = lntmp.tile([128, KC, S], BF16, name="tmp",
                                         tag="ap")
                        nc.vector.tensor_add(
                            out=tmp[:], in0=xTb3[:, b, :, 0:S],
                            in1=M_sb[:, 0:1, :].broadcast_to((128, KC, S)))
                        for kp in range(KC // 2):
                            nc.vector.tensor_mul(
                                out=x3[b][:, 2 * kp:2 * kp + 2, 0:S],
                                in0=tmp[:, 2 * kp:2 * kp + 2, :],
                                in1=R_sb[:, 0:1, :]
                                .broadcast_to((128, 2, S)))
                        v_proj(b)

                    # the 4 batches' tail token (j=256), packed col groups
                    ps = ppv.tile([128, 2, 512], F32, name="vps_t", tag="vps")
                    for kp in range(KC // 2):
                        for n in range(2):
                            nc.tensor.matmul(
                                ps[0:97, n, :],
                                xtail[:, 2 * kp:2 * kp + 2, 0:97],
                                vw_sb[:, kp, :, n * 512:(n + 1) * 512],
                                start=(kp == 0), stop=(kp == 3),
                                perf_mode=DR)
                    if with_vbias:
                        nc.vector.scalar_tensor_tensor(
                            out=vt_t[0:97, :, 64:128],
                            in0=ps[0:97, :, :], scalar=dq_v,
                            in1=vb_sb[0:97, :], op0=ALU.mult, op1=ALU.add)
                    else:
                        nc.vector.tensor_scalar_mul(
                            out=vt_t[0:97, :, 64:128],
                            in0=ps[0:97, :, :], scalar1=dq_v)

                # ========= Q/K + attention (per head-pair chunk) =====
                with ExitStack() as ph:
                    qt_p = ph.enter_context(tc.tile_pool(name="qt", bufs=2))
                    kt_p = ph.enter_context(tc.tile_pool(name="kt", bufs=2))
                    wqk_p = ph.enter_context(
                        tc.tile_pool(name="wqk", bufs=2))
                    qw_sb = wqk_p.tile([128, MC_E, KC, 128], FP8,
                                       name="qw_sb", tag="wqk")
                    nc.scalar.dma_start(out=qw_sb[:], in_=qw_d[:])
                    kw_sb = wqk_p.tile([128, MC_E, KC, 128], FP8,
                                       name="kw_sb", tag="wqk")
                    nc.scalar.dma_start(out=kw_sb[:], in_=kw_d[:])
                    e_p = ph.enter_context(tc.tile_pool(name="ep", bufs=9))
                    rs_p = ph.enter_context(tc.tile_pool(name="rsp", bufs=4))
                    if with_mask:
                        msk_p = ph.enter_context(
                            tc.tile_pool(name="mskp", bufs=3 * B_LOC))
                    # one unified attention PSUM pool: every tile is
                    # [128, 2, 512] (2 banks), ring depth 4 = all 8 banks.
                    # With 3 allocs per softmax batch (sp, sp, cp) plus the
                    # interleaved QK/tail allocs, ring slots stagger so
                    # cp(b+1) no longer waits on cp(b)'s recip/muls.
                    pat = ph.enter_context(
                        tc.tile_pool(name="pat", bufs=4, space="PSUM"))
                    if with_mask:
                        msk = {}
                        for b in range(B_LOC):
                            for jc, (j0, jcs) in enumerate(JC):
                                mt = msk_p.tile([128, S], F32, name="mt",
                                                tag="mt")
                                nc.sync.dma_start(
                                    out=mt[0:jcs, :],
                                    in_=mskT_d[b, j0:j0 + jcs, :])
                                msk[(b, jc)] = mt

                    # software-pipelined: QK projection chunks of ec+1 are
                    # emitted BETWEEN the softmax groups of ec so the PE
                    # always has dense matmul work during exp/copy waits
                    # (keeps the HAM clock gate at 8/8 through attention)
                    qte_d, kte_d, ett_d = {}, {}, {}

                    def qk_chunk(ec, ci):
                        proj, half = divmod(ci, 2)
                        w_sb, b_sb, opool, otd = (
                            (qw_sb, qb_sb, qt_p, qte_d) if proj == 0
                            else (kw_sb, kb_sb, kt_p, kte_d))
                        if half == 0:
                            otd[ec] = opool.tile([128, B_LOC, PT], BF16,
                                                 name="qk", tag="qk")
                        ot = otd[ec]
                        pss = pat.tile([128, 2, 512], F32,
                                       name="pqk", tag="pat")
                        for kp in range(KC // 2):
                            for bb in range(2):
                                b = half * 2 + bb
                                nc.tensor.matmul(
                                    pss[:, bb, 0:S],
                                    w_sb[:, ec, 2 * kp:2 * kp + 2, :],
                                    x3[b][:, 2 * kp:2 * kp + 2, 0:S],
                                    start=(kp == 0), stop=(kp == 3),
                                    perf_mode=DR)
                        for bb in range(2):
                            b = half * 2 + bb
                            if with_qkbias:
                                nc.vector.tensor_scalar_add(
                                    out=ot[:, b, 0:S],
                                    in0=pss[:, bb, 0:S],
                                    scalar1=b_sb[:, ec:ec + 1])
                            elif proj == 1:
                                # K copies on ACT to balance DVE load
                                nc.scalar.copy(out=ot[:, b, 0:S],
                                               in_=pss[:, bb, 0:S])
                            else:
                                nc.vector.tensor_copy(
                                    out=ot[:, b, 0:S],
                                    in_=pss[:, bb, 0:S])

                    def tail_chunk(ec):
                        # tail key (j=256) for all 4 batches: packed into
                        # array col groups 32b / row groups 64*hi.
                        qte, kte = qte_d[ec], kte_d[ec]
                        ps_t = pat.tile([128, 2, 512], F32, name="ps_t",
                                        tag="pat")
                        et_t = e_p.tile([128, 2, S], BF16, name="et_t",
                                        tag="et")
                        for hi in range(2):
                            p0 = hi * 64
                            for b in range(B_LOC):
                                nc.tensor.matmul(
                                    ps_t[32 * b:32 * b + 1, hi, 0:S],
                                    kte[p0:p0 + 64, b, 256:257],
                                    qte[p0:p0 + 64, b, 0:S],
                                    start=True, stop=True,
                                    tile_position=(p0, 32 * b))
                            if with_mask:
                                for b in range(B_LOC):
                                    nc.vector.tensor_add(
                                        out=ps_t[32 * b:32 * b + 1, hi, 0:S],
                                        in0=ps_t[32 * b:32 * b + 1, hi, 0:S],
                                        in1=msk[(b, 2)][0:1, :])
                        nc.scalar.activation(out=et_t[0:97, :, :],
                                             in_=ps_t[0:97, :, 0:S],
                                             func=AF.Exp, scale=dq_qk)
                        ett_d[ec] = et_t

                    def sp_chunk(ec, b):
                        qte, kte = qte_d[ec], kte_d[ec]
                        ets = []
                        for jc, (j0, jcs) in enumerate(JC[:2]):
                            sp = pat.tile([128, 2, 512], F32,
                                          name="sp", tag="pat")
                            for hi in range(2):
                                p0 = hi * 64
                                nc.tensor.matmul(
                                    sp[0:jcs, hi, 0:S],
                                    kte[p0:p0 + 64, b, j0:j0 + jcs],
                                    qte[p0:p0 + 64, b, 0:S],
                                    start=True, stop=True)
                            if with_mask:
                                for hi in range(2):
                                    nc.vector.tensor_add(
                                        out=sp[0:jcs, hi, 0:S],
                                        in0=sp[0:jcs, hi, 0:S],
                                        in1=msk[(b, jc)][0:jcs, :])
                            et = e_p.tile([128, 2, S], BF16,
                                          name="et", tag="et2")
                            nc.scalar.activation(
                                out=et[0:jcs, :, :],
                                in_=sp[0:jcs, :, 0:S], func=AF.Exp,
                                scale=dq_qk)
                            ets.append(et)
                        return ets

                    def ctx_chunk(ec, b, ets):
                        et_t = ett_d[ec]
                        cp = pat.tile([128, 2, 512], F32,
                                      name="cp", tag="pat")
                        for hi in range(2):
                            h = 2 * ec + hi
                            for jc, (j0, jcs) in enumerate(JC[:2]):
                                nc.tensor.matmul(
                                    cp[0:128, hi, 0:S],
                                    v_tiles[(b, jc)][0:jcs, h, :],
                                    ets[jc][0:jcs, hi, :],
                                    start=(jc == 0), stop=False)
                            nc.tensor.matmul(
                                cp[0:128, hi, 0:S],
                                v_tiles[(b, 2)][32 * b:32 * b + 1, h, :],
                                et_t[32 * b:32 * b + 1, hi, :],
                                start=False, stop=True,
                                tile_position=(32 * b, 0))
                        rst = rs_p.tile([64, 2, S], F32,
                                        name="rst", tag="rst")
                        nc.vector.reciprocal_approx_fast(
                            out=rst[0:64, :, :],
                            in_=cp[0:64, :, 0:S])
                        for hi in range(2):
                            nc.vector.tensor_mul(
                                out=ctx3[b][hi * 64:hi * 64 + 64,
                                            ec, 0:S],
                                in0=cp[64:128, hi, 0:S],
                                in1=rst[0:64, hi, :])

                    for ci in range(4):
                        qk_chunk(0, ci)
                    tail_chunk(0)
                    for ec in range(MC_E):
                        for b in range(B_LOC):
                            ets = sp_chunk(ec, b)
                            if ec + 1 < MC_E:
                                qk_chunk(ec + 1, b)
                            ctx_chunk(ec, b, ets)
                        if ec + 1 < MC_E:
                            tail_chunk(ec + 1)

            # ==== out projection (+identity residual) fused with LN2; fc1 ====
            f1o_p = top.enter_context(
                tc.tile_pool(name="f1o", bufs=MC_I, side="right"))
            f1o = []
            wf2_p = top.enter_context(tc.tile_pool(name="wf2", bufs=3))
            wf2_pre = []
            with ExitStack() as oph:
                wo_p = oph.enter_context(tc.tile_pool(name="wo", bufs=1))
                wf1_p = oph.enter_context(tc.tile_pool(name="wf1", bufs=1))
                sq2_p = oph.enter_context(tc.tile_pool(name="sqp2",
                                                       bufs=B_LOC))
                rows2 = oph.enter_context(tc.tile_pool(name="rows2", bufs=24))
                rsb2_p = oph.enter_context(tc.tile_pool(name="rsb2", bufs=4))
                lntmp2 = oph.enter_context(tc.tile_pool(name="lntmp2",
                                                        bufs=2))
                ppo = oph.enter_context(
                    tc.tile_pool(name="ppo", bufs=2, space="PSUM"))
                pbc2 = oph.enter_context(
                    tc.tile_pool(name="pbc2", bufs=2, space="PSUM"))

                wo_sb = wo_p.tile([128, MC_E, KC, 128], FP8, name="wo",
                                  tag="wo")
                nc.gpsimd.dma_start(out=wo_sb[:], in_=ow_d[:])
                wf1_sb = wf1_p.tile([128, MC_I, KC, 128], FP8, name="wf1",
                                    tag="wf1")
                nc.gpsimd.dma_start(out=wf1_sb[:], in_=f1w_d[:])
                # prefetch the first fc2 weight tiles during o_proj/fc1 so
                # fc2 starts the moment fc1 drains
                for m in range(3):
                    wt = wf2_p.tile([128, MC_I, 128], BF16, name="wf2",
                                    tag="wf2")
                    nc.gpsimd.dma_start(out=wt[:], in_=f2w_d[m, :, :, :])
                    wf2_pre.append(wt)

                def o_proj(b):
                    for m in range(MC_E):
                        ps = ppo.tile([128, 512], F32, name="po", tag="po")
                        for kp in range(KC // 2):
                            nc.tensor.matmul(
                                ps[:, 0:S],
                                wo_sb[:, m, 2 * kp:2 * kp + 2, :],
                                ctx3[b][:, 2 * kp:2 * kp + 2, 0:S],
                                start=(kp == 0), stop=False,
                                perf_mode=DR)
                        # residual: += (1/dq_o) * I @ x  (exact: dq_o = 2^-k)
                        nc.tensor.matmul(
                            ps[:, 0:S], ident_sb[:], xTb3[:, b, m, 0:S],
                            start=False, stop=True)
                        if with_obias:
                            nc.vector.scalar_tensor_tensor(
                                out=ht3[:, m, b, 0:S], in0=ps[:, 0:S],
                                scalar=dq_o, in1=ob_sb[:, m:m + 1]
                                .broadcast_to((128, S)),
                                op0=ALU.mult, op1=ALU.add)
                        else:
                            nc.scalar.mul(out=ht3[:, m, b, 0:S],
                                          in_=ps[:, 0:S], mul=dq_o)

                def sqb2_emit(b):
                    sq = sq2_p.tile([128, KC, S], BF16, name="sqb2",
                                    tag="sqb2")
                    nc.vector.tensor_mul(out=sq[:], in0=ht3[:, :, b, 0:S],
                                         in1=ht3[:, :, b, 0:S])
                    return sq

                def ln2_norm(b, rws2b):
                    M_sb, R_sb = emit_ln_bcast(pbc2, rsb2_p, *rws2b)
                    tmp = lntmp2.tile([128, KC, S], BF16, name="tmp2",
                                      tag="ap2")
                    nc.vector.tensor_add(
                        out=tmp[:], in0=ht3[:, :, b, 0:S],
                        in1=M_sb[:, 0:1, :].broadcast_to((128, KC, S)))
                    for kp in range(KC // 2):
                        nc.vector.tensor_mul(
                            out=xln2_3[b][:, 2 * kp:2 * kp + 2, 0:S],
                            in0=tmp[:, 2 * kp:2 * kp + 2, :],
                            in1=R_sb[:, 0:1, :].broadcast_to((128, 2, S)))

                def fc1_half(half, ppf1, m_lo=0, m_hi=MC_I):
                    for m in range(m_lo, m_hi):
                        ps = ppf1.tile([128, 2, 512], F32, name="pf1",
                                       tag="pf1")
                        for kp in range(KC // 2):
                            for bb in range(2):
                                b = half * 2 + bb
                                nc.tensor.matmul(
                                    ps[:, bb, 0:S],
                                    wf1_sb[:, m, 2 * kp:2 * kp + 2, :],
                                    xln2_3[b][:, 2 * kp:2 * kp + 2, 0:S],
                                    start=(kp == 0), stop=(kp == 3),
                                    perf_mode=DR)
                        if half == 0:
                            o = f1o_p.tile([128, NT], BF16, name="f1o",
                                           tag="f1o")
                            f1o.append(o)
                        else:
                            o = f1o[m]
                        nc.scalar.activation(
                            out=o[:, half * 2 * S:(half + 1) * 2 * S],
                            in_=ps[:, :, 0:S],
                            func=AF.Gelu_apprx_tanh,
                            bias=f1b_sb[:, m:m + 1],
                            scale=dq_f1)

                # pipeline: o_proj per batch; LN2 stages slotted between;
                # fc1 half-passes as soon as their xln2 batches are ready
                o_proj(0)
                sq0 = sqb2_emit(0)
                o_proj(1)
                sq1 = sqb2_emit(1)
                sqs = {0: sq0, 1: sq1}
                st3s2 = {}
                with ExitStack() as stat2_ph:
                    pstat2 = stat2_ph.enter_context(
                        tc.tile_pool(name="pstat2", bufs=2, space="PSUM"))

                    def stats2(pair):
                        blo, bhi = 2 * pair, 2 * pair + 1
                        st = pstat2.tile([128, 512], F32, name="st2",
                                         tag="stat2")
                        for k in range(KC):
                            nc.tensor.matmul(st[0:1, 0:S], ones_col[:],
                                             ht3[:, k, blo, 0:S],
                                             start=(k == 0), stop=(k == KC - 1))
                            nc.tensor.matmul(st[32:33, 0:S], ones_col[:],
                                             sqs[blo][:, k, :],
                                             start=(k == 0), stop=(k == KC - 1),
                                             tile_position=(0, 32))
                            nc.tensor.matmul(st[64:65, 0:S], ones_col[:],
                                             ht3[:, k, bhi, 0:S],
                                             start=(k == 0), stop=(k == KC - 1),
                                             tile_position=(0, 64))
                            nc.tensor.matmul(st[96:97, 0:S], ones_col[:],
                                             sqs[bhi][:, k, :],
                                             start=(k == 0), stop=(k == KC - 1),
                                             tile_position=(0, 96))
                        st3s2[blo] = (st, 0, 32)
                        st3s2[bhi] = (st, 64, 96)

                    stats2(0)
                    rws2 = {b: emit_ln_rows(rows2, st3s2[b]) for b in (0, 1)}
                    ln2_norm(0, rws2[0])
                    ln2_norm(1, rws2[1])
                    o_proj(2)
                    sqs[2] = sqb2_emit(2)
                    o_proj(3)
                    sqs[3] = sqb2_emit(3)
                    stats2(1)
                    for b in (2, 3):
                        rws2[b] = emit_ln_rows(rows2, st3s2[b])
                # pstat2 closed -> banks free for fc1
                ppf1 = oph.enter_context(
                    tc.tile_pool(name="ppf1", bufs=2, space="PSUM"))
                # first fc1 m-chunk (needs only batches 0/1) keeps the PE
                # fed while the LN2 rows/normalize for batches 2/3 drain
                fc1_half(0, ppf1, 0, 8)
                ln2_norm(2, rws2[2])
                ln2_norm(3, rws2[3])
                fc1_half(0, ppf1, 8, MC_I)
                fc1_half(1, ppf1)
        # ctx3 closed

        # ================= fc2 =====================================
        with ExitStack() as ph:
            ppf2 = ph.enter_context(
                tc.tile_pool(name="ppf2", bufs=2, space="PSUM"))
            out_p = ph.enter_context(tc.tile_pool(name="outp", bufs=3))
            for m in range(MC_E):
                if m < 3:
                    wt = wf2_pre[m]
                else:
                    wt = wf2_p.tile([128, MC_I, 128], BF16, name="wf2",
                                    tag="wf2")
                    nc.gpsimd.dma_start(out=wt[:], in_=f2w_d[m, :, :, :])
                ps = ppf2.tile([128, B_LOC, 512], F32, name="pf2", tag="pf2")
                for b in range(B_LOC):
                    for k in range(MC_I):
                        nc.tensor.matmul(
                            ps[:, b, 0:S], wt[:, k, :],
                            f1o[k][:, b * S:(b + 1) * S],
                            start=(k == 0), stop=(k == MC_I - 1))
                o = out_p.tile([128, B_LOC, S], F32, name="oo", tag="oo")
                nc.vector.scalar_tensor_tensor(
                    out=o[:], in0=ps[:, :, 0:S], scalar=f2b_sb[:, m:m + 1],
                    in1=ht3[:, m, :, 0:S], op0=ALU.add, op1=ALU.add)
                nc.sync.dma_start(out=outT_d[m * 128:(m + 1) * 128, :],
                                  in_=o[:])

    nc.compile()
    return nc


FP8_NP = ml_dtypes.float8_e4m3fn


def _q8(W, s):
    """Quantize W*s to e4m3 (clipped to TRN max normal 240)."""
    return np.clip(np.asarray(W, np.float32) * s, -240, 240).astype(FP8_NP)


def _pack_lhsT8(W, s):
    """W [M, K] (out, in) -> [128, M/128, K/128, 128] fp8 with
    [p, m, k, j] = W[m*128+j, k*128+p]*s (partition-major lhsT tiles:
    each partition row is one contiguous DRAM run -> 128-descriptor DMA)."""
    W = np.asarray(W, np.float32)
    M, K = W.shape
    A = W.reshape(M // 128, 128, K // 128, 128)
    return _q8(np.ascontiguousarray(A.transpose(3, 0, 2, 1)), s)


def _pack_lhsT(W):
    """bf16 variant of _pack_lhsT8 (no scale)."""
    W = np.asarray(W, np.float32)
    M, K = W.shape
    A = W.reshape(M // 128, 128, K // 128, 128)
    return np.ascontiguousarray(A.transpose(0, 3, 2, 1)).astype(ml_dtypes.bfloat16)


def _pack_pbias(b):
    """b [M] -> [128, M/128] f32 per-partition bias columns."""
    return np.ascontiguousarray(np.asarray(b, np.float32).reshape(-1, 128).T)


def _wscale(W):
    """Power-of-2 scale with max |W*s| in (60, 120]."""
    m = max(np.abs(np.asarray(W, np.float32)).max(), 1e-30)
    return float(2.0 ** np.floor(np.log2(120.0 / m)))


def kernel(hidden_states, attention_mask, causal_attention_mask,
           ln1_w, ln1_b, q_w, q_b, k_w, k_b, v_w, v_b, o_w, o_b,
           ln2_w, ln2_b, fc1_w, fc1_b, fc2_w, fc2_b):
    global LAST_EXEC_NS
    from concourse.bass_utils import run_bass_kernel_spmd

    hs = np.asarray(hidden_states, np.float32)
    msk = (np.asarray(attention_mask, np.float32)
           + np.asarray(causal_attention_mask, np.float32))
    with_mask = bool(np.any(msk))

    ln1_w = np.asarray(ln1_w, np.float32); ln1_b = np.asarray(ln1_b, np.float32)
    ln2_w = np.asarray(ln2_w, np.float32); ln2_b = np.asarray(ln2_b, np.float32)
    q_w = np.asarray(q_w, np.float32); q_b = np.asarray(q_b, np.float32)
    k_w = np.asarray(k_w, np.float32); k_b = np.asarray(k_b, np.float32)
    v_w = np.asarray(v_w, np.float32); v_b = np.asarray(v_b, np.float32)
    o_w = np.asarray(o_w, np.float32); o_b = np.asarray(o_b, np.float32)
    fc1_w = np.asarray(fc1_w, np.float32); fc1_b = np.asarray(fc1_b, np.float32)
    fc2_w = np.asarray(fc2_w, np.float32); fc2_b = np.asarray(fc2_b, np.float32)

    scale = D ** -0.5
    # fold LN1 scale/bias into Q/K/V, and the softmax scale into Q
    qw_eff = (q_w * ln1_w[None, :]) * scale
    qb_eff = (q_b + q_w @ ln1_b) * scale
    kw_eff = k_w * ln1_w[None, :]
    kb_eff = k_b + k_w @ ln1_b
    vw_eff = v_w * ln1_w[None, :]
    vb_eff = v_b + v_w @ ln1_b
    # fold LN2 into fc1
    f1w_eff = fc1_w * ln2_w[None, :]
    f1b_eff = fc1_b + fc1_w @ ln2_b

    # fp8 weight scales (power-of-2; LN activations pre-scaled by S_X)
    s_wq = _wscale(qw_eff)
    s_wk = _wscale(kw_eff)
    s_wv = _wscale(vw_eff)
    s_wo = _wscale(o_w)
    s_wf1 = _wscale(f1w_eff)
    dq_qk = 1.0 / (S_X * S_X * s_wq * s_wk)
    dq_v = 1.0 / (S_X * s_wv)
    dq_o = 1.0 / (S_CTX * s_wo)
    dq_f1 = 1.0 / (S_X * s_wf1)

    # vw: [E_in, E_out] grouped into k-pairs -> [128, KC/2, 2, E] fp8
    vw_t = np.ascontiguousarray(vw_eff.T.reshape(KC, 128, E))
    vw_pk = np.ascontiguousarray(
        vw_t.reshape(KC // 2, 2, 128, E).transpose(2, 0, 1, 3))

    base = {
        "qw": _pack_lhsT8(qw_eff, s_wq),
        "kw": _pack_lhsT8(kw_eff, s_wk),
        "vw": _q8(vw_pk, s_wv),
        "ow": _pack_lhsT8(o_w, s_wo),
        "f1w": _pack_lhsT8(f1w_eff, s_wf1),
        "f2w": _pack_lhsT(fc2_w),
        "ident": np.ascontiguousarray(
            (np.eye(128, dtype=np.float32) / dq_o)
            .astype(ml_dtypes.bfloat16)),
        "qb": _pack_pbias(qb_eff * (S_X * s_wq)),
        "kb": _pack_pbias(kb_eff * (S_X * s_wk)),
        "vb": np.ascontiguousarray(vb_eff[None, :].astype(np.float32)),
        "ob": _pack_pbias(o_b),
        "f1b": _pack_pbias(f1b_eff),
        "f2b": _pack_pbias(fc2_b),
    }

    with_vbias = bool(np.any(vb_eff))
    with_qkbias = bool(np.any(qb_eff)) or bool(np.any(kb_eff))
    with_obias = bool(np.any(o_b))
    key = (with_mask, with_vbias, with_qkbias, with_obias,
           dq_v, dq_qk, dq_o, dq_f1)
    if key not in _cache:
        _cache[key] = _build(with_mask, with_vbias, with_qkbias, with_obias,
                             dq_v, dq_qk, dq_o, dq_f1)
    nc = _cache[key]

    in_maps = []
    for c in range(N_CORES):
        # [128, B_LOC, KC, PT]: partition-major, b-major
        xp = np.zeros((128, B_LOC, KC, PT), np.float32)
        for b in range(B_LOC):
            xb = hs[c * B_LOC + b]                      # [S, E]
            xp[:, b, :, 0:S] = xb.T.reshape(KC, 128, S).transpose(1, 0, 2)
        m = dict(base)
        m["xTb"] = np.ascontiguousarray(xp).astype(ml_dtypes.bfloat16)
        if with_mask:
            m["mskT"] = np.ascontiguousarray(
                msk[c * B_LOC:(c + 1) * B_LOC, 0].transpose(0, 2, 1)
                / dq_qk)
        in_maps.append(m)

    res = run_bass_kernel_spmd(nc, in_maps, core_ids=list(range(N_CORES)),
                               trace=TRACE)
    LAST_EXEC_NS = res.exec_time_ns

    outs = []
    for c in range(N_CORES):
        oT = res.results[c]["outT"]          # [E, NT] f32
        outs.append(np.ascontiguousarray(oT.T).reshape(B_LOC, S, E))
    return np.concatenate(outs, axis=0)
